# revision 16
# baseline (speedup 1.0000x reference)
"""Causal self-attention on 8 TRN2 NeuronCores.

Problem: B=4, S=2048, D=1024, H=16 heads (hd=64), fp32 in/out.
  qkv = x @ w_qkv + b_qkv ; causal softmax attention ; y @ w_out + b_out

Sharding (tensor-parallel over heads x data-parallel over batch):
  core c -> batch b = c//2, head-group hg = c%2 (8 heads each).
  Host unshards: out[b] = partial[2b] + partial[2b+1] + b_out.

Device kernel v2 -- single software-pipelined phase:
  - projections (qk per token-block, v per key-tile) are issued as small
    "units" interleaved into the attention instruction stream so the PE
    never drains while ACT chases exps; attention for query-block a only
    needs q/k token-blocks <= a, so qk units are staged by token block.
  - scores for 2 key-blocks of one head share a 2-bank PSUM tile and one
    exp instruction (halves ACT instruction overhead); diagonal blocks
    keep per-block strip exps with a 128-wide triangular mask add.
  - softmax normalization never touches ACT (which therefore only ever
    runs Exp -- zero activation-table swaps): ones column FIRST in v_aug
    puts the rowsum on PSUM partition 0 -> DVE reciprocal_approx_fast
    (PSUM->SBUF) -> gpsimd partition_broadcast -> DVE multiply evicts
    psum -> partition-shift DMA into yT.
  - out-projection for token-block a is issued right after attention(a),
    filling the PE while ACT finishes; output DMA'd as bf16 (host sums
    partials in fp32).
"""

import os
import sys

for _p in ("/root/.axon_site/_ro/trn_rl_repo", "/opt/trn_rl_repo"):
    if os.path.isdir(_p) and _p not in sys.path:
        sys.path.append(_p)

import ml_dtypes
import numpy as np

import concourse.bass as bass  # noqa: F401
import concourse.mybir as mybir
import concourse.tile as tile
from concourse import bacc
from concourse.bass_utils import run_bass_kernel_spmd

B, S, D, H = 4, 2048, 1024, 16
HD = 64
HPC = 8          # heads per core
NPAIR = HPC // 2
KO = D // 128    # contraction chunks over D
KT = S // 128    # key tiles
TB = S // 512    # token blocks
ATT_SCALE = 1.0 / np.sqrt(HD)
NEG = -1.0e30

F32 = mybir.dt.float32
BF16 = mybir.dt.bfloat16
NPBF16 = ml_dtypes.bfloat16


def build_nc(S_=S):
    nc = bacc.Bacc(None)
    xT_d = nc.dram_tensor("xT", [D, S_], BF16, kind="ExternalInput")
    wqk_d = nc.dram_tensor("wqk", [D, NPAIR, 2, 128], BF16, kind="ExternalInput")
    bqk_d = nc.dram_tensor("bqk", [128, NPAIR, 2], F32, kind="ExternalInput")
    wv_d = nc.dram_tensor("wv", [D, HPC * HD], BF16, kind="ExternalInput")
    bv_d = nc.dram_tensor("bv", [128, HPC * HD], F32, kind="ExternalInput")
    wout_d = nc.dram_tensor("wout", [HPC * HD, D], BF16, kind="ExternalInput")
    mask_d = nc.dram_tensor("mask", [128, 128], F32, kind="ExternalInput")
    out_d = nc.dram_tensor("out", [S_, D], BF16, kind="ExternalOutput")

    with tile.TileContext(nc) as tc, nc.allow_low_precision("bf16 matmul operands"):
        with (
            tc.tile_pool(name="const", bufs=1) as constp,
            tc.tile_pool(name="patt", bufs=4) as patt,
            tc.tile_pool(name="pnorm", bufs=4) as pnorm,
            tc.tile_pool(name="postage", bufs=3) as postage,
            tc.tile_pool(name="psS", bufs=2, space="PSUM") as psS,
            tc.tile_pool(name="psY", bufs=2, space="PSUM") as psY,
            tc.tile_pool(name="psA", bufs=2, space="PSUM") as psA,
        ):
            # ---- constants / inputs (DMA order = need order) ----
            mask_sb = constp.tile([128, 128], F32)
            nc.sync.dma_start(mask_sb[:], mask_d[:])
            bqk_sb = constp.tile([128, NPAIR, 2], F32)
            nc.sync.dma_start(bqk_sb[:], bqk_d[:])
            bv_sb = constp.tile([128, HPC * HD], F32)
            nc.sync.dma_start(bv_sb[:], bv_d[:])

            xT = constp.tile([128, KO, S_], BF16)
            xr = xT_d.rearrange("(ko p) t -> p ko t", p=128)
            wqk_sb = constp.tile([128, KO, NPAIR, 2, 128], BF16)
            wr = wqk_d.rearrange("(ko p) r c2 c -> p ko r c2 c", p=128)
            for i in range(4):
                nc.sync.dma_start(xT[:, 2 * i : 2 * i + 2, :], xr[:, 2 * i : 2 * i + 2, :])
                nc.sync.dma_start(wqk_sb[:, 2 * i : 2 * i + 2], wr[:, 2 * i : 2 * i + 2])
            wv_sb = constp.tile([128, KO, HPC * HD], BF16)
            nc.sync.dma_start(wv_sb[:], wv_d.rearrange("(ko p) c -> p ko c", p=128))
            wout_sb = constp.tile([128, NPAIR, D], BF16)
            nc.sync.dma_start(wout_sb[:], wout_d.rearrange("(cc p) c -> p cc c", p=128))

            # v with ones column (col 64) -> PV rowsum on PSUM partition 64
            # (engine partition accesses must start 0/64-aligned); col 65 pad
            vaug = constp.tile([128, KT, HPC, 66], BF16)
            nc.gpsimd.memset(vaug[:, :, :, 64], 1.0)
            qkT = constp.tile([128, NPAIR, 2, S_], BF16)   # [.., 0,..]=q, [.., 1,..]=k
            yT = constp.tile([128, NPAIR, S_], BF16)

            # ---- projection units (issued lazily, staged by token block) ----
            def qk_unit(tb, pr, cqk):
                def run():
                    ps = psA.tile([128, 512], F32, tag="psA")
                    for k in range(KO):
                        nc.tensor.matmul(
                            ps,
                            wqk_sb[:, k, pr, cqk, :],
                            xT[:, k, tb * 512 : (tb + 1) * 512],
                            start=(k == 0),
                            stop=(k == KO - 1),
                        )
                    nc.vector.tensor_scalar_add(
                        qkT[:, pr, cqk, tb * 512 : (tb + 1) * 512],
                        ps[:],
                        bqk_sb[:, pr, cqk : cqk + 1],
                    )
                return run

            def v_unit(tt):
                def run():
                    ps = psA.tile([128, 512], F32, tag="psA")
                    for k in range(KO):
                        nc.tensor.matmul(
                            ps,
                            xT[:, k, tt * 128 : (tt + 1) * 128],
                            wv_sb[:, k, :],
                            start=(k == 0),
                            stop=(k == KO - 1),
                        )
                    nc.vector.tensor_tensor(
                        vaug[:, tt, :, 0:64],
                        ps[:].rearrange("p (h d) -> p h d", h=HPC),
                        bv_sb[:].rearrange("p (h d) -> p h d", h=HPC),
                        mybir.AluOpType.add,
                    )
                return run

            # out-proj chains double as PE filler, popped on demand where the
            # attention stream would otherwise stall (exp waits, norm drains)
            op_chains = []

            def op_chain(tt, nh):
                def run():
                    ps = psA.tile([128, 512], F32, tag="psA")
                    for cc in range(NPAIR):
                        nc.tensor.matmul(
                            ps,
                            yT[:, cc, tt * 128 : (tt + 1) * 128],
                            wout_sb[:, cc, nh * 512 : (nh + 1) * 512],
                            start=(cc == 0),
                            stop=(cc == NPAIR - 1),
                        )
                    ot = postage.tile([128, 512], BF16, tag="ot")
                    nc.vector.tensor_copy(ot[:], ps[:])
                    nc.sync.dma_start(
                        out_d[tt * 128 : (tt + 1) * 128, nh * 512 : (nh + 1) * 512],
                        ot[:],
                    )
                return run

            def pop_filler():
                if op_chains:
                    op_chains.pop(0)()

            # ---- normalization (no ACT): bcast rowsum, fast reciprocal ----
            def make_norm(pr, a, psy):
                def run():
                    dsts = a * 512
                    for h01 in range(2):
                        # rowsum on PSUM partition 64: DVE fast-reciprocal in
                        # place (lane 64), DMA partition-shift to row 0,
                        # gpsimd broadcast, DVE multiply evicts psum rows
                        # 0..63 into yT (h1 via staging + shift DMA).
                        ri = pnorm.tile([65, 512], F32, tag="ri")
                        nc.vector.tensor_copy(ri[64:65, :], psy[h01][64:65, :])
                        ri0 = pnorm.tile([1, 512], F32, tag="ri0")
                        nc.sync.dma_start(ri0[:], ri[64:65, :])
                        rv = pnorm.tile([1, 512], F32, tag="rv")
                        nc.vector.reciprocal_approx_fast(rv[:], ri0[:])
                        bc = pnorm.tile([64, 512], F32, tag="bc")
                        nc.gpsimd.partition_broadcast(bc[:], rv[:])
                        if h01 == 0:
                            nc.vector.tensor_tensor(
                                yT[0:64, pr, dsts : dsts + 512],
                                psy[h01][0:64, :],
                                bc[:],
                                mybir.AluOpType.mult,
                            )
                        else:
                            stg = pnorm.tile([64, 512], BF16, tag="stg")
                            nc.vector.tensor_tensor(
                                stg[:],
                                psy[h01][0:64, :],
                                bc[:],
                                mybir.AluOpType.mult,
                            )
                            nc.sync.dma_start(
                                yT[64:128, pr, dsts : dsts + 512], stg[:]
                            )
                return run

            # ---- fused attention + out-projection, a-outer ----
            # v units at block start, qk units just-in-time per pair (their
            # 8 matmuls bridge the norm-chain drain of the previous pair);
            # out-proj chains are the only filler left during the last block,
            # so reserve them for its pair boundaries.
            for a in range(TB):
                for tt in range(4 * a, 4 * a + 4):
                    v_unit(tt)()
                for pr in range(NPAIR):
                    qk_unit(a, pr, 0)()
                    qk_unit(a, pr, 1)()
                    psy = [
                        psY.tile([65, 512], F32, tag="psY", name=f"psy{h}")
                        for h in range(2)
                    ]
                    nj = 4 * a + 4
                    for g in range(2 * a + 2):
                        jj = (2 * g, 2 * g + 1)
                        diag = 2 * g >= 4 * a
                        for h01 in range(2):
                            lo, hi = h01 * 64, h01 * 64 + 64
                            pss = psS.tile([128, 1024], F32, tag="psS")
                            for i, j in enumerate(jj):
                                o = 128 * j - 512 * a
                                oo = max(o, 0)
                                nc.tensor.matmul(
                                    pss[:, i * 512 + oo : (i + 1) * 512],
                                    qkT[lo:hi, pr, 1, j * 128 : (j + 1) * 128],
                                    qkT[lo:hi, pr, 0, a * 512 + oo : (a + 1) * 512],
                                    start=True,
                                    stop=True,
                                )
                            att = patt.tile([128, 1024], BF16, tag="att")
                            if diag:
                                for i, j in enumerate(jj):
                                    o = 128 * j - 512 * a
                                    nc.vector.tensor_tensor(
                                        pss[:, i * 512 + o : i * 512 + o + 128],
                                        pss[:, i * 512 + o : i * 512 + o + 128],
                                        mask_sb[:],
                                        mybir.AluOpType.add,
                                    )
                                for i, j in enumerate(jj):
                                    o = 128 * j - 512 * a
                                    nc.scalar.activation(
                                        att[:, i * 512 + o : (i + 1) * 512],
                                        pss[:, i * 512 + o : (i + 1) * 512],
                                        mybir.ActivationFunctionType.Exp,
                                        scale=float(ATT_SCALE),
                                    )
                            else:
                                nc.scalar.activation(
                                    att[:],
                                    pss[:],
                                    mybir.ActivationFunctionType.Exp,
                                    scale=float(ATT_SCALE),
                                )
                            for i, j in enumerate(jj):
                                o = 128 * j - 512 * a
                                oo = max(o, 0)
                                nc.tensor.matmul(
                                    psy[h01][:, oo:512],
                                    vaug[:, j, 2 * pr + h01, 0:65],
                                    att[:, i * 512 + oo : (i + 1) * 512],
                                    start=(j == 0),
                                    stop=(j == nj - 1),
                                    skip_group_check=True,
                                )
                            if a < TB - 1:
                                pop_filler()
                    # normalize right away: next pair's qk units / fillers
                    # keep the PE busy while this drains (psY bufs=2 cannot
                    # hold two pairs' accumulators)
                    make_norm(pr, a, psy)()
                    if a == TB - 1 and pr < NPAIR - 1:
                        pop_filler()
                        pop_filler()
                # out-projection for token block a becomes filler material
                for tt in range(4 * a, 4 * a + 4):
                    for nh in range(2):
                        op_chains.append(op_chain(tt, nh))
            for c in op_chains:
                c()

    nc.finalize()
    return nc


def make_host_inputs(x, w_qkv, b_qkv, w_out, b_out, S_=S):
    """Build the 8 per-core input maps (host-side shard/pack/cast)."""
    x = np.asarray(x, dtype=np.float32)
    w_qkv = np.asarray(w_qkv, dtype=np.float32)
    b_qkv = np.asarray(b_qkv, dtype=np.float32)
    w_out = np.asarray(w_out, dtype=np.float32)

    mask = np.where(
        np.arange(128)[None, :] >= np.arange(128)[:, None], 0.0, NEG
    ).astype(np.float32)

    per_hg = {}
    for hg in range(2):
        wqk = np.empty((D, NPAIR, 2, 128), np.float32)
        bqk = np.empty((128, NPAIR, 2), np.float32)
        for p in range(NPAIR):
            h0, h1 = hg * HPC + 2 * p, hg * HPC + 2 * p + 1
            wqk[:, p, 0, 0:64] = w_qkv[:, h0 * HD : (h0 + 1) * HD]
            wqk[:, p, 0, 64:128] = w_qkv[:, h1 * HD : (h1 + 1) * HD]
            wqk[:, p, 1, 0:64] = w_qkv[:, D + h0 * HD : D + (h0 + 1) * HD]
            wqk[:, p, 1, 64:128] = w_qkv[:, D + h1 * HD : D + (h1 + 1) * HD]
            bqk[0:64, p, 0] = b_qkv[h0 * HD : (h0 + 1) * HD]
            bqk[64:128, p, 0] = b_qkv[h1 * HD : (h1 + 1) * HD]
            bqk[0:64, p, 1] = b_qkv[D + h0 * HD : D + (h0 + 1) * HD]
            bqk[64:128, p, 1] = b_qkv[D + h1 * HD : D + (h1 + 1) * HD]
        wv = w_qkv[:, 2 * D + hg * 512 : 2 * D + (hg + 1) * 512]
        bv = np.broadcast_to(
            b_qkv[2 * D + hg * 512 : 2 * D + (hg + 1) * 512], (128, 512)
        ).copy()
        wout = w_out[hg * 512 : (hg + 1) * 512, :]
        per_hg[hg] = dict(
            wqk=np.ascontiguousarray(wqk.astype(NPBF16)),
            bqk=bqk,
            wv=np.ascontiguousarray(wv.astype(NPBF16)),
            bv=bv,
            wout=np.ascontiguousarray(wout.astype(NPBF16)),
        )

    xT_by_b = [
        np.ascontiguousarray(x[b, :S_].T.astype(NPBF16)) for b in range(B)
    ]
    in_maps = []
    for c in range(8):
        b, hg = c // 2, c % 2
        m = dict(per_hg[hg])
        m["xT"] = xT_by_b[b]
        m["mask"] = mask
        in_maps.append(m)
    return in_maps


_NC_CACHE = {}


def _get_nc(S_=S):
    if S_ not in _NC_CACHE:
        _NC_CACHE[S_] = build_nc(S_)
    return _NC_CACHE[S_]


def kernel(x, w_qkv, b_qkv, w_out, b_out):
    x = np.asarray(x, dtype=np.float32)
    b_out = np.asarray(b_out, dtype=np.float32)
    in_maps = make_host_inputs(x, w_qkv, b_qkv, w_out, b_out)
    nc = _get_nc()
    res = run_bass_kernel_spmd(nc, in_maps, list(range(8))).results
    out = np.empty((B, S, D), np.float32)
    for b in range(B):
        out[b] = (
            res[2 * b]["out"].astype(np.float32)
            + res[2 * b + 1]["out"].astype(np.float32)
            + b_out[None, :]
        )
    return out
